# revision 4
# baseline (speedup 1.0000x reference)
"""Global top-k (k=10%) binary masks for two 8192x2048-sized f32 tensors,
distributed over 8 TRN2 NeuronCores.

Algorithm (per tensor, global over all shards):
  1. Each core loads its row-shard into SBUF, takes |x| in place.
  2. While loading, counts elements >= t0 and >= t0+w (t0 = the N(0,1)
     90% |quantile|) -> per-core counts -> one tiny AllReduce.
  3. Newton step with the empirical slope (C(t0)-C(t0+w))/w gives t1;
     a verification count at t1 + second AllReduce + secant step gives t2.
  4. mask = (|x| >= t2), streamed back out.
The mask differs from exact argsort-top-k only in a few boundary elements
(secant residual is ~a few counts out of 1.68M kept).
"""

import sys
import numpy as np

sys.path.insert(0, "/opt/trn_rl_repo")

from concourse import bass, bacc, mybir, tile  # noqa: E402
from concourse import bass_utils  # noqa: E402

P = 128
TILE_F = 2048
N_TILES = 16  # per core: 8 down-tiles + 8 up-tiles (up reshaped to 2048-wide)
N_CORES = 8

N_FULL = 8192 * 2048          # elements per tensor (global)
J = int(0.9 * N_FULL)         # int((1-k)*n) as in reference
M_TARGET = float(N_FULL - J)  # number of kept (=1) entries per tensor
T0 = 1.6448536269514722       # Phi^-1(0.95): 90% quantile of |N(0,1)|
W = 0.01                      # slope-probe offset

F32 = mybir.dt.float32
ALU = mybir.AluOpType
ACT = mybir.ActivationFunctionType


def build_nc():
    nc = bacc.Bacc(None, target_bir_lowering=False, debug=False, num_devices=N_CORES)

    down = nc.declare_dram_parameter("down", [1024, 2048], F32, isOutput=False)
    up = nc.declare_dram_parameter("up", [256, 8192], F32, isOutput=False)
    out_down = nc.declare_dram_parameter("out_down", [1024, 2048], F32, isOutput=True)
    out_up = nc.declare_dram_parameter("out_up", [256, 8192], F32, isOutput=True)

    # Uniform [N_TILES, 128, 2048] views of both shards (row-major preserving).
    down_r = down[:, :].rearrange("(a p) m -> a p m", p=P)
    up_r = up[:, :].rearrange("r (b m) -> (r b) m", b=4).rearrange(
        "(a p) m -> a p m", p=P
    )
    out_down_r = out_down[:, :].rearrange("(a p) m -> a p m", p=P)
    out_up_r = out_up[:, :].rearrange("r (b m) -> (r b) m", b=4).rearrange(
        "(a p) m -> a p m", p=P
    )

    rg = [list(range(N_CORES))]

    with tile.TileContext(nc) as tc:
        with (
            tc.tile_pool(name="data", bufs=1) as data_pool,
            tc.tile_pool(name="scr", bufs=4) as scr_pool,
            tc.tile_pool(name="stats", bufs=1) as stats_pool,
            tc.tile_pool(name="psum", bufs=1, space="PSUM") as psum_pool,
            tc.tile_pool(name="dram", bufs=1, space="DRAM") as dram_pool,
        ):
            data_tiles = [
                data_pool.tile([P, TILE_F], F32, tag=f"data{k}", name=f"data{k}")
                for k in range(N_TILES)
            ]
            cnt01 = stats_pool.tile([P, 2 * N_TILES], F32, tag="cnt01")
            cnt2 = stats_pool.tile([P, N_TILES], F32, tag="cnt2")
            ones = stats_pool.tile([P, 1], F32, tag="ones")
            nc.vector.memset(ones[:], 1.0)

            # ---- Phase A: load, |x|, count at t0 and t0+w ----
            for k in range(N_TILES):
                src = down_r[k] if k < 8 else up_r[k - 8]
                d = data_tiles[k]
                nc.sync.dma_start(out=d[:], in_=src)
                nc.scalar.activation(d[:], d[:], ACT.Abs)
                s0 = scr_pool.tile([P, TILE_F], F32, tag="scr")
                nc.vector.tensor_scalar(
                    out=s0[:], in0=d[:], scalar1=T0, scalar2=0.0,
                    op0=ALU.is_ge, op1=ALU.add, accum_out=cnt01[:, k : k + 1],
                )
                s1 = scr_pool.tile([P, TILE_F], F32, tag="scr")
                nc.vector.tensor_scalar(
                    out=s1[:], in0=d[:], scalar1=T0 + W, scalar2=0.0,
                    op0=ALU.is_ge, op1=ALU.add, accum_out=cnt01[:, N_TILES + k : N_TILES + k + 1],
                )

            # Reduce partition axis via matmul with ones, then free axis.
            pc1 = psum_pool.tile([1, 2 * N_TILES], F32, tag="pc1")
            nc.tensor.matmul(pc1[:], ones[:], cnt01[:], start=True, stop=True)
            cc_loc1 = stats_pool.tile([1, 8], F32, tag="cc_loc1")
            nc.vector.memset(cc_loc1[:], 0.0)
            # layout: [C0_down, C0_up, C0w_down, C0w_up, 0...]
            nc.vector.reduce_sum(cc_loc1[0:1, 0:1], pc1[0:1, 0:8], axis=mybir.AxisListType.X)
            nc.vector.reduce_sum(cc_loc1[0:1, 1:2], pc1[0:1, 8:16], axis=mybir.AxisListType.X)
            nc.vector.reduce_sum(cc_loc1[0:1, 2:3], pc1[0:1, 16:24], axis=mybir.AxisListType.X)
            nc.vector.reduce_sum(cc_loc1[0:1, 3:4], pc1[0:1, 24:32], axis=mybir.AxisListType.X)

            cc_in1 = dram_pool.tile([1, 8], F32, tag="cc_in1")
            cc_out1 = dram_pool.tile([1, 8], F32, tag="cc_out1")
            nc.sync.dma_start(out=cc_in1[:], in_=cc_loc1[:])
            nc.gpsimd.collective_compute(
                "AllReduce", ALU.add, replica_groups=rg,
                ins=[cc_in1[:].opt()], outs=[cc_out1[:].opt()],
            )
            g1 = stats_pool.tile([1, 8], F32, tag="g1")
            nc.sync.dma_start(out=g1[:], in_=cc_out1[:])

            # ---- Newton step with empirical slope ----
            # slope_unit = (C0 - C0w)  (> 0), actual slope = slope_unit / W
            # t1 = t0 + (C0 - M) * W / slope_unit
            err0 = stats_pool.tile([1, 2], F32, tag="err0")
            diff = stats_pool.tile([1, 2], F32, tag="diff")
            rec = stats_pool.tile([1, 2], F32, tag="rec")
            step0 = stats_pool.tile([1, 2], F32, tag="step0")
            t1 = stats_pool.tile([1, 2], F32, tag="t1")
            nc.vector.tensor_scalar(
                out=err0[:], in0=g1[0:1, 0:2], scalar1=-M_TARGET, scalar2=None,
                op0=ALU.add,
            )
            nc.vector.scalar_tensor_tensor(
                out=diff[:], in0=g1[0:1, 0:2], scalar=1.0, in1=g1[0:1, 2:4],
                op0=ALU.mult, op1=ALU.subtract,
            )
            nc.vector.reciprocal(rec[:], diff[:])
            nc.vector.tensor_tensor(
                out=step0[:], in0=err0[:], in1=rec[:], op=ALU.mult
            )
            nc.vector.tensor_scalar(
                out=t1[:], in0=step0[:], scalar1=W, scalar2=T0,
                op0=ALU.mult, op1=ALU.add,
            )

            # broadcast t1 across partitions via DRAM bounce
            thr1_d = dram_pool.tile([1, 2], F32, tag="thr1_d")
            nc.sync.dma_start(out=thr1_d[:], in_=t1[:])
            thr1 = stats_pool.tile([P, 2], F32, tag="thr1")
            nc.sync.dma_start(out=thr1[:], in_=thr1_d[0:1, 0:2].partition_broadcast(P))

            # ---- Phase B: verification count at t1 ----
            for k in range(N_TILES):
                col = 0 if k < 8 else 1
                s = scr_pool.tile([P, TILE_F], F32, tag="scr")
                nc.vector.tensor_scalar(
                    out=s[:], in0=data_tiles[k][:], scalar1=thr1[:, col : col + 1],
                    scalar2=0.0, op0=ALU.is_ge, op1=ALU.add, accum_out=cnt2[:, k : k + 1],
                )
            pc2 = psum_pool.tile([1, N_TILES], F32, tag="pc2")
            nc.tensor.matmul(pc2[:], ones[:], cnt2[:], start=True, stop=True)
            cc_loc2 = stats_pool.tile([1, 8], F32, tag="cc_loc2")
            nc.vector.memset(cc_loc2[:], 0.0)
            nc.vector.reduce_sum(cc_loc2[0:1, 0:1], pc2[0:1, 0:8], axis=mybir.AxisListType.X)
            nc.vector.reduce_sum(cc_loc2[0:1, 1:2], pc2[0:1, 8:16], axis=mybir.AxisListType.X)

            cc_in2 = dram_pool.tile([1, 8], F32, tag="cc_in2")
            cc_out2 = dram_pool.tile([1, 8], F32, tag="cc_out2")
            nc.sync.dma_start(out=cc_in2[:], in_=cc_loc2[:])
            nc.gpsimd.collective_compute(
                "AllReduce", ALU.add, replica_groups=rg,
                ins=[cc_in2[:].opt()], outs=[cc_out2[:].opt()],
            )
            g2 = stats_pool.tile([1, 8], F32, tag="g2")
            nc.sync.dma_start(out=g2[:], in_=cc_out2[:])

            # ---- secant step: t2 = t1 + (C1 - M) * W / slope_unit ----
            err1 = stats_pool.tile([1, 2], F32, tag="err1")
            step1 = stats_pool.tile([1, 2], F32, tag="step1")
            t2 = stats_pool.tile([1, 2], F32, tag="t2")
            nc.vector.tensor_scalar(
                out=err1[:], in0=g2[0:1, 0:2], scalar1=-M_TARGET, scalar2=None,
                op0=ALU.add,
            )
            nc.vector.tensor_tensor(
                out=step1[:], in0=err1[:], in1=rec[:], op=ALU.mult
            )
            nc.vector.scalar_tensor_tensor(
                out=t2[:], in0=step1[:], scalar=W, in1=t1[:],
                op0=ALU.mult, op1=ALU.add,
            )
            thr2_d = dram_pool.tile([1, 2], F32, tag="thr2_d")
            nc.sync.dma_start(out=thr2_d[:], in_=t2[:])
            thr2 = stats_pool.tile([P, 2], F32, tag="thr2")
            nc.sync.dma_start(out=thr2[:], in_=thr2_d[0:1, 0:2].partition_broadcast(P))

            # ---- Phase C: apply mask, store ----
            for k in range(N_TILES):
                col = 0 if k < 8 else 1
                mt = scr_pool.tile([P, TILE_F], F32, tag="scr")
                nc.vector.tensor_scalar(
                    out=mt[:], in0=data_tiles[k][:], scalar1=thr2[:, col : col + 1],
                    scalar2=None, op0=ALU.is_ge,
                )
                dst = out_down_r[k] if k < 8 else out_up_r[k - 8]
                nc.sync.dma_start(out=dst, in_=mt[:])

    nc.compile()
    return nc


_CACHED = {}


def kernel(down_mask: np.ndarray, up_mask: np.ndarray, _want_trace: bool = False):
    down_mask = np.ascontiguousarray(down_mask, dtype=np.float32)
    up_mask = np.ascontiguousarray(up_mask, dtype=np.float32)
    assert down_mask.shape == (8192, 2048) and up_mask.shape == (2048, 8192)

    if "nc" not in _CACHED:
        _CACHED["nc"] = build_nc()
    nc = _CACHED["nc"]

    in_maps = []
    for i in range(N_CORES):
        in_maps.append(
            {
                "down": down_mask[i * 1024 : (i + 1) * 1024, :],
                "up": up_mask[i * 256 : (i + 1) * 256, :],
            }
        )

    res = bass_utils.run_bass_kernel_spmd(
        nc, in_maps, core_ids=list(range(N_CORES)), trace=_want_trace
    )
    outs = res.results
    down_out = np.concatenate([outs[i]["out_down"] for i in range(N_CORES)], axis=0)
    up_out = np.concatenate([outs[i]["out_up"] for i in range(N_CORES)], axis=0)
    if _want_trace:
        return (down_out, up_out), res
    return down_out, up_out
